# revision 1
# baseline (speedup 1.0000x reference)
"""4-bit column-block-quantized linear (ColBlockQuantizedLinear) on 8 Trainium2 NeuronCores.

Reference computation:
    w[n, k] = (nibble(quant_weight)[n, k] - zeros[n]) * scales[n]     n<11008, k<4096
    out[b, s, n] = sum_k inp[b, s, k] * w[n, k]                        inp: [4, 2048, 4096] f32

Strategy (column-parallel, per sharding hint):
  - Shard out_features N=11008 = 8*1376 across 8 cores; replicate inp.
  - Host-side layout prep only: transpose/permute inp to k-major bf16, cast packed
    weights int32->uint8 and transpose to [k/2, n] per core, row-sums of inp,
    broadcast scale rows.
  - On-chip per core: unpack nibbles into resident SBUF weight tiles holding the
    RAW 4-bit values (exact in bf16), then a dense bf16 matmul accumulating
    psum[m, n] += xT[k, m].T @ Q[k, n] over 32 k-tiles. Dequant is folded into
    the f32 PSUM eviction:  out = psum * s[n] + (-s[n]*z[n]) * rowsum[m],
    which is exact up to the bf16 rounding of the activations.
  - A short burst of dummy matmuls warms the PE (HAM un-throttle) while the
    first activation tiles and weights stream in.
  - Host concatenates per-core outputs along N.
"""

import sys

for _p in ("/opt/trn_rl_repo", "/opt/pypackages"):
    if _p not in sys.path:
        sys.path.append(_p)

import numpy as np
import ml_dtypes

import concourse.bass as bass
import concourse.mybir as mybir
import concourse.tile as tile
from concourse import bacc

# Problem constants (hardcoded per harness contract)
B, S, K = 4, 2048, 4096
M = B * S                  # 8192 tokens
N = 11008                  # out features
NCORES = 8
NPC = N // NCORES          # per-core out features (1376)
KP = K // 2                # packed k rows (2048)
P = 128


def _nchunks(npc, wide=False):
    step = 1024 if wide else 512
    return [(i, min(step, npc - i)) for i in range(0, npc, step)]


def build_nc(m=M, kp=KP, npc=NPC, mg=512, warmup=120, wide=False):
    """Build the per-core Bass program. m tokens, kp packed-k rows, npc out cols,
    mg tokens per m-group (DMA granule)."""
    ktp = kp // P              # packed k tiles (16)
    kt_n = 2 * ktp             # unpacked k tiles (32)
    ngroups = m // mg
    mbs = mg // P              # m-blocks per group
    chunks = _nchunks(npc, wide)

    nc = bacc.Bacc("TRN2", target_bir_lowering=False, debug=False)
    xt_d = nc.dram_tensor("xt", [kt_n, P, m], mybir.dt.bfloat16, kind="ExternalInput")
    qwt_d = nc.dram_tensor("qwt", [ktp, P, npc], mybir.dt.uint8, kind="ExternalInput")
    s_d = nc.dram_tensor("s32", [P, npc], mybir.dt.float32, kind="ExternalInput")
    nb_d = nc.dram_tensor("nb32", [P, npc], mybir.dt.float32, kind="ExternalInput")
    rs_d = nc.dram_tensor("rs", [P, m // P], mybir.dt.float32, kind="ExternalInput")
    out_d = nc.dram_tensor("out", [m, npc], mybir.dt.float32, kind="ExternalOutput")

    with tile.TileContext(nc) as tc:
        with (
            tc.tile_pool(name="const", bufs=1) as const_pool,
            tc.tile_pool(name="stage", bufs=2) as stage_pool,
            tc.tile_pool(name="w", bufs=1) as w_pool,
            tc.tile_pool(name="x", bufs=2) as x_pool,
            tc.tile_pool(name="o", bufs=2) as o_pool,
            tc.tile_pool(name="ps", bufs=2, space="PSUM") as ps_pool,
            tc.tile_pool(name="wps", bufs=1, space="PSUM") as warm_ps_pool,
        ):
            s32t = const_pool.tile([P, npc], mybir.dt.float32, tag="s32t")
            nb32t = const_pool.tile([P, npc], mybir.dt.float32, tag="nb32t")
            rs_t = const_pool.tile([P, m // P], mybir.dt.float32, tag="rs_t")
            # PE warmup: flip the HAM clock gate to 8/8 while DMAs/unpack run.
            if warmup:
                wsrc = const_pool.tile([P, 256], mybir.dt.bfloat16, tag="wsrc")
                nc.vector.memset(wsrc[:], 0.0)
                wp = warm_ps_pool.tile([P, 256], mybir.dt.float32, tag="wp")
                for _ in range(warmup):
                    nc.tensor.matmul(wp[:], wsrc[:, :P], wsrc[:], start=True, stop=True)

            xg0 = x_pool.tile([P, kt_n, mg], mybir.dt.bfloat16, tag="xg")

            # Unpack raw nibbles into resident SBUF tiles (values 0..15, exact
            # in bf16). W[kt] for kt in [0, ktp) = low nibbles (even k),
            # [ktp, 2ktp) = high nibbles (odd k).
            w_tiles = [
                w_pool.tile([P, npc], mybir.dt.bfloat16, name=f"W{kt}", tag=f"W{kt}")
                for kt in range(kt_n)
            ]
            for kt in range(ktp):
                q = stage_pool.tile([P, npc], mybir.dt.uint8, tag="q")
                nc.sync.dma_start(q[:], qwt_d[kt])
                # bitwise must be dtype-preserving; do it 2 packed bytes per
                # lane-op via a u16 view (masks are byte-symmetric), then
                # cast u8->bf16 on ScalarE
                lo8 = stage_pool.tile([P, npc], mybir.dt.uint8, tag="lo8")
                hi8 = stage_pool.tile([P, npc], mybir.dt.uint8, tag="hi8")
                nc.vector.tensor_scalar(
                    lo8[:].bitcast(mybir.dt.uint16), q[:].bitcast(mybir.dt.uint16),
                    0x0F0F, None, op0=mybir.AluOpType.bitwise_and,
                )
                nc.vector.tensor_scalar(
                    hi8[:].bitcast(mybir.dt.uint16), q[:].bitcast(mybir.dt.uint16),
                    4, 0x0F0F,
                    op0=mybir.AluOpType.logical_shift_right,
                    op1=mybir.AluOpType.bitwise_and,
                )
                nc.scalar.copy(w_tiles[kt][:], lo8[:])
                nc.scalar.copy(w_tiles[ktp + kt][:], hi8[:])
                # two g0 activation DMAs per weight tile: interleaved so the
                # first m-group lands early without starving the weight queue
                nc.sync.dma_start(xg0[:, 2 * kt, :], xt_d[2 * kt, :, 0:mg])
                nc.sync.dma_start(xg0[:, 2 * kt + 1, :], xt_d[2 * kt + 1, :, 0:mg])

            # scale rows are first needed at the first eviction (~50us in):
            # keep their 1.4MB out of the pre-mb0 DMA critical path
            nc.sync.dma_start(s32t[:], s_d[:])
            nc.sync.dma_start(nb32t[:], nb_d[:])
            nc.sync.dma_start(rs_t[:], rs_d[:])

            # Main matmul loop: m-groups of `mg` tokens, 128-token m-blocks.
            # k-tiles consumed in unpack-completion order (low_t, high_t).
            kt_order = [t + h * ktp for t in range(ktp) for h in (0, 1)]
            for g in range(ngroups):
                if g == 0:
                    xg = xg0
                else:
                    xg = x_pool.tile([P, kt_n, mg], mybir.dt.bfloat16, tag="xg")
                    for kt in range(kt_n):
                        nc.sync.dma_start(
                            xg[:, kt, :], xt_d[kt, :, g * mg:(g + 1) * mg]
                        )
                for mb in range(mbs):
                    mbi = g * mbs + mb
                    ps = ps_pool.tile([P, npc], mybir.dt.float32, tag="ps")
                    for i, kt in enumerate(kt_order):
                        lhsT = xg[:, kt, mb * P:(mb + 1) * P]
                        for (n0, nw) in chunks:
                            nc.tensor.matmul(
                                ps[:, n0:n0 + nw], lhsT, w_tiles[kt][:, n0:n0 + nw],
                                start=(i == 0), stop=(i == kt_n - 1),
                            )
                    # eviction (ACT, keeps PSUM-read pattern cheap for PE),
                    # then dequant on SBUF:  out = psum * s + (-s*z) * rowsum[m]
                    ot = o_pool.tile([P, npc], mybir.dt.float32, tag="ot")
                    nc.scalar.copy(ot[:], ps[:])
                    nc.vector.tensor_tensor(
                        ot[:], ot[:], s32t[:], op=mybir.AluOpType.mult
                    )
                    nc.vector.scalar_tensor_tensor(
                        ot[:], nb32t[:], rs_t[:, mbi:mbi + 1], ot[:],
                        op0=mybir.AluOpType.mult, op1=mybir.AluOpType.add,
                    )
                    m0 = g * mg + mb * P
                    nc.sync.dma_start(out_d[m0:m0 + P, :], ot[:])

    nc.compile()
    return nc


def prep_inputs(inp, quant_weight, scales, zeros, ncores=NCORES, npc=NPC):
    """Host-side sharding/layout: returns in_maps list for run_bass_kernel_spmd."""
    m = inp.shape[0] * inp.shape[1]
    k = inp.shape[2]
    kp = k // 2
    ktp = kp // P

    x = np.asarray(inp, dtype=np.float32).reshape(m, k)
    x3 = x.reshape(m, kp, 2)
    # xt rows: kt in [0, ktp) -> even k (low nibble), [ktp, 2ktp) -> odd k (high)
    xt_even = np.ascontiguousarray(x3[:, :, 0].T).astype(ml_dtypes.bfloat16)
    xt_odd = np.ascontiguousarray(x3[:, :, 1].T).astype(ml_dtypes.bfloat16)
    xt = np.concatenate(
        [xt_even.reshape(ktp, P, m), xt_odd.reshape(ktp, P, m)], axis=0
    )  # [2*ktp, P, m] bf16

    # rowsum of the exact activations, for the zero-point correction term
    rs = x.sum(axis=1, dtype=np.float64).astype(np.float32)  # [m]
    rs_host = np.ascontiguousarray(rs.reshape(m // P, P).T)  # [P, m//P]

    n = quant_weight.shape[0]
    assert n == ncores * npc, (n, ncores, npc)
    qw8 = np.asarray(quant_weight).astype(np.uint8)
    s_all = np.asarray(scales, dtype=np.float32).reshape(-1)
    z_all = np.asarray(zeros, dtype=np.float32).reshape(-1)
    nb_all = -(s_all * z_all)

    in_maps = []
    for c in range(ncores):
        sl = slice(c * npc, (c + 1) * npc)
        qwt_c = np.ascontiguousarray(qw8[sl].T).reshape(ktp, P, npc)
        s_c = np.ascontiguousarray(np.broadcast_to(s_all[sl], (P, npc)))
        nb_c = np.ascontiguousarray(np.broadcast_to(nb_all[sl], (P, npc)))
        in_maps.append(
            {"xt": xt, "qwt": qwt_c, "s32": s_c, "nb32": nb_c, "rs": rs_host}
        )
    return in_maps


_NC_CACHE = {}


def _get_nc():
    if "nc" not in _NC_CACHE:
        _NC_CACHE["nc"] = build_nc()
    return _NC_CACHE["nc"]


def kernel(inp, quant_weight, scales, zeros):
    from concourse.bass_utils import run_bass_kernel_spmd

    nc = _get_nc()
    in_maps = prep_inputs(inp, quant_weight, scales, zeros)
    res = run_bass_kernel_spmd(nc, in_maps, list(range(NCORES)))
    out = np.concatenate([res.results[c]["out"] for c in range(NCORES)], axis=1)
    return np.ascontiguousarray(out).reshape(B, S, N)



# revision 3
# speedup vs baseline: 1.8831x; 1.8831x over previous
"""4-bit column-block-quantized linear (ColBlockQuantizedLinear) on 8 Trainium2 NeuronCores.

Reference computation:
    w[n, k] = (nibble(quant_weight)[n, k] - zeros[n]) * scales[n]     n<11008, k<4096
    out[b, s, n] = sum_k inp[b, s, k] * w[n, k]                        inp: [4, 2048, 4096] f32

Strategy (column-parallel, per sharding hint):
  - Shard out_features N=11008 = 8*1376 across 8 cores; replicate inp.
  - fp8 double-pumped matmul (MatmulPerfMode.DoubleRow, 2x the bf16 PE rate):
    both operands are float8e4 (e4m3). Host ships activations rounded to e4m3
    and weights expanded to CENTERED nibbles (q - 7.5), which are exact in
    e4m3 (values +-0.5 .. +-7.5).
  - Centering is the accuracy trick: the fp8 rounding error of the
    activations couples to the matmul weights, so using (q - 7.5) instead of
    raw q (RMS 4.6 vs 8.8) cuts the error ~1.9x. The 7.5 shift is folded
    back exactly at eviction through the f64-accurate host row-sums:
        out = psum * s[n] + s[n]*(7.5 - z[n]) * rowsum[m]
    Measured l2 rel err ~1.7e-2 (vs 3.2e-2 uncentered).
  - K = 4096 = 16 pairs x (2 planes x 128); DoubleRow contracts both planes
    of a pair per instruction: lhsT = x8[128, 2, 128m], moving =
    w8[128, 2, <=256n] (moving free dim capped at 512).
  - A short burst of dummy matmuls warms the PE (HAM un-throttle) while the
    first activation tiles and weights stream in.
  - Host concatenates per-core outputs along N.
"""

import sys

for _p in ("/opt/trn_rl_repo", "/opt/pypackages"):
    if _p not in sys.path:
        sys.path.append(_p)

import numpy as np
import ml_dtypes

import concourse.bass as bass
import concourse.mybir as mybir
import concourse.tile as tile
from concourse import bacc

# Problem constants (hardcoded per harness contract)
B, S, K = 4, 2048, 4096
M = B * S                  # 8192 tokens
N = 11008                  # out features
NCORES = 8
NPC = N // NCORES          # per-core out features (1376)
P = 128
KPAIRS = K // (2 * P)      # 16 pairs of k-planes (256 k each)
CENTER = 7.5               # nibble centering; q - 7.5 is exact in e4m3


def _nchunks(npc, cw=256):
    return [(i, min(cw, npc - i)) for i in range(0, npc, cw)]


def build_nc(m=M, npc=NPC, mg=512, warmup=120, cw=256):
    """Build the per-core Bass program. m tokens, npc out cols, mg tokens per
    m-group (DMA granule), cw psum chunk width (<=256 for DoubleRow)."""
    ngroups = m // mg
    mbs = mg // P              # m-blocks per group
    chunks = _nchunks(npc, cw)
    f8 = mybir.dt.float8e4

    nc = bacc.Bacc("TRN2", target_bir_lowering=False, debug=False)
    x8_d = nc.dram_tensor("x8", [KPAIRS, P, 2, m], f8, kind="ExternalInput")
    w8_d = nc.dram_tensor("w8", [KPAIRS, P, 2, npc], f8, kind="ExternalInput")
    s_d = nc.dram_tensor("s32", [P, npc], mybir.dt.float32, kind="ExternalInput")
    cb_d = nc.dram_tensor("cb32", [P, npc], mybir.dt.float32, kind="ExternalInput")
    rs_d = nc.dram_tensor("rs", [P, m // P], mybir.dt.float32, kind="ExternalInput")
    out_d = nc.dram_tensor("out", [m, npc], mybir.dt.float32, kind="ExternalOutput")

    with tile.TileContext(nc) as tc:
        with (
            tc.tile_pool(name="const", bufs=1) as const_pool,
            tc.tile_pool(name="w", bufs=1) as w_pool,
            tc.tile_pool(name="x", bufs=2) as x_pool,
            tc.tile_pool(name="o", bufs=2) as o_pool,
            tc.tile_pool(name="ps", bufs=2, space="PSUM") as ps_pool,
            tc.tile_pool(name="wps", bufs=1, space="PSUM") as warm_ps_pool,
        ):
            s32t = const_pool.tile([P, npc], mybir.dt.float32, tag="s32t")
            cb32t = const_pool.tile([P, npc], mybir.dt.float32, tag="cb32t")
            rs_t = const_pool.tile([P, m // P], mybir.dt.float32, tag="rs_t")
            # PE warmup: flip the HAM clock gate to 8/8 while DMAs run.
            if warmup:
                wsrc = const_pool.tile([P, 256], mybir.dt.bfloat16, tag="wsrc")
                nc.vector.memset(wsrc[:], 0.0)
                wp = warm_ps_pool.tile([P, 256], mybir.dt.float32, tag="wp")
                for _ in range(warmup):
                    nc.tensor.matmul(wp[:], wsrc[:, :P], wsrc[:], start=True, stop=True)

            xg0 = x_pool.tile([P, KPAIRS, 2, mg], f8, tag="xg")

            # Resident fp8 weight tiles, one per k-pair, already centered on host.
            w_tiles = [
                w_pool.tile([P, 2, npc], f8, name=f"W{t}", tag=f"W{t}")
                for t in range(KPAIRS)
            ]
            # interleave weight and first-group activation DMAs so pair t of
            # both lands early, letting mb0's accumulation start ASAP
            for t in range(KPAIRS):
                nc.sync.dma_start(w_tiles[t][:], w8_d[t])
                nc.sync.dma_start(xg0[:, t, :, :], x8_d[t, :, :, 0:mg])

            # scale rows are first needed at the first eviction: keep their
            # 1.4MB out of the pre-mb0 DMA critical path
            nc.sync.dma_start(s32t[:], s_d[:])
            nc.sync.dma_start(cb32t[:], cb_d[:])
            nc.sync.dma_start(rs_t[:], rs_d[:])

            # Main matmul loop: m-groups of `mg` tokens, 128-token m-blocks.
            for g in range(ngroups):
                if g == 0:
                    xg = xg0
                else:
                    xg = x_pool.tile([P, KPAIRS, 2, mg], f8, tag="xg")
                    for t in range(KPAIRS):
                        nc.sync.dma_start(
                            xg[:, t, :, :], x8_d[t, :, :, g * mg:(g + 1) * mg]
                        )
                for mb in range(mbs):
                    mbi = g * mbs + mb
                    ps = ps_pool.tile([P, npc], mybir.dt.float32, tag="ps")
                    for t in range(KPAIRS):
                        lhsT = xg[:, t, :, mb * P:(mb + 1) * P]
                        for (n0, nw) in chunks:
                            # start=True marks the whole 2KB PSUM bank (the
                            # zero region) pending-zero, so only the FIRST
                            # chunk in each bank may issue it; the bank's
                            # second chunk inherits the marking and its first
                            # (start=False) write still overwrites.
                            bank_first = (n0 % 512) == 0
                            nc.tensor.matmul(
                                ps[:, n0:n0 + nw], lhsT,
                                w_tiles[t][:, :, n0:n0 + nw],
                                start=(t == 0 and bank_first),
                                stop=(t == KPAIRS - 1),
                                perf_mode=mybir.MatmulPerfMode.DoubleRow,
                                skip_group_check=(not bank_first),
                            )
                    # eviction (ACT), then dequant on SBUF:
                    #   out = psum * s + s*(7.5 - z) * rowsum[m]
                    ot = o_pool.tile([P, npc], mybir.dt.float32, tag="ot")
                    nc.scalar.copy(ot[:], ps[:])
                    nc.vector.tensor_tensor(
                        ot[:], ot[:], s32t[:], op=mybir.AluOpType.mult
                    )
                    nc.vector.scalar_tensor_tensor(
                        ot[:], cb32t[:], rs_t[:, mbi:mbi + 1], ot[:],
                        op0=mybir.AluOpType.mult, op1=mybir.AluOpType.add,
                    )
                    m0 = g * mg + mb * P
                    nc.sync.dma_start(out_d[m0:m0 + P, :], ot[:])

    nc.compile()
    return nc


def prep_inputs(inp, quant_weight, scales, zeros, ncores=NCORES, npc=NPC):
    """Host-side sharding/layout: returns in_maps list for run_bass_kernel_spmd."""
    m = inp.shape[0] * inp.shape[1]
    k = inp.shape[2]

    x = np.asarray(inp, dtype=np.float32).reshape(m, k)
    # x8[t, p, i, tok] = e4m3(x[tok, 256t + 2p + i]): plane i=0 even k (low
    # nibble), i=1 odd k (high nibble), paired per DoubleRow instruction
    x8 = np.ascontiguousarray(
        x.reshape(m, KPAIRS, P, 2).astype(ml_dtypes.float8_e4m3)
        .transpose(1, 2, 3, 0)
    )

    # rowsum of the exact activations, for the center/zero correction term
    rs = x.sum(axis=1, dtype=np.float64).astype(np.float32)  # [m]
    rs_host = np.ascontiguousarray(rs.reshape(m // P, P).T)  # [P, m//P]

    n = quant_weight.shape[0]
    assert n == ncores * npc, (n, ncores, npc)
    qw8 = np.asarray(quant_weight).astype(np.uint8)          # [N, k//2]
    lo = (qw8 & 15).astype(np.float32) - CENTER              # even k
    hi = (qw8 >> 4).astype(np.float32) - CENTER              # odd k
    s_all = np.asarray(scales, dtype=np.float32).reshape(-1)
    z_all = np.asarray(zeros, dtype=np.float32).reshape(-1)
    cb_all = s_all * (CENTER - z_all)

    in_maps = []
    for c in range(ncores):
        sl = slice(c * npc, (c + 1) * npc)
        # w8[t, p, i, n]: centered nibbles, exact in e4m3
        wc = np.stack([lo[sl].T, hi[sl].T], axis=1)          # [k//2, 2, npc]
        wc = np.ascontiguousarray(
            wc.reshape(KPAIRS, P, 2, npc).astype(ml_dtypes.float8_e4m3)
        )
        s_c = np.ascontiguousarray(np.broadcast_to(s_all[sl], (P, npc)))
        cb_c = np.ascontiguousarray(np.broadcast_to(cb_all[sl], (P, npc)))
        in_maps.append(
            {"x8": x8, "w8": wc, "s32": s_c, "cb32": cb_c, "rs": rs_host}
        )
    return in_maps


_NC_CACHE = {}


def _get_nc():
    if "nc" not in _NC_CACHE:
        _NC_CACHE["nc"] = build_nc()
    return _NC_CACHE["nc"]


def kernel(inp, quant_weight, scales, zeros):
    from concourse.bass_utils import run_bass_kernel_spmd

    nc = _get_nc()
    in_maps = prep_inputs(inp, quant_weight, scales, zeros)
    res = run_bass_kernel_spmd(nc, in_maps, list(range(NCORES)))
    out = np.concatenate([res.results[c]["out"] for c in range(NCORES)], axis=1)
    return np.ascontiguousarray(out).reshape(B, S, N)


# revision 5
# speedup vs baseline: 1.9522x; 1.0367x over previous
"""4-bit column-block-quantized linear (ColBlockQuantizedLinear) on 8 Trainium2 NeuronCores.

Reference computation:
    w[n, k] = (nibble(quant_weight)[n, k] - zeros[n]) * scales[n]     n<11008, k<4096
    out[b, s, n] = sum_k inp[b, s, k] * w[n, k]                        inp: [4, 2048, 4096] f32

Strategy (column-parallel, per sharding hint):
  - Shard out_features N=11008 = 8*1376 across 8 cores; replicate inp.
  - fp8 double-pumped matmul (MatmulPerfMode.DoubleRow, 2x the bf16 PE rate):
    both operands are float8e4 (e4m3). Host ships activations rounded to e4m3
    and weights expanded to CENTERED nibbles (q - 7.5), which are exact in
    e4m3 (values +-0.5 .. +-7.5).
  - Centering is the accuracy trick: the fp8 rounding error of the
    activations couples to the matmul weights, so using (q - 7.5) instead of
    raw q (RMS 4.6 vs 8.8) cuts the error ~1.9x. The 7.5 shift is folded
    back exactly at eviction through the f64-accurate host row-sums:
        out = psum * s[n] + s[n]*(7.5 - z[n]) * rowsum[m]
    Measured l2 rel err ~1.7e-2 (vs 3.2e-2 uncentered).
  - K = 4096 = 16 pairs x (2 planes x 128); DoubleRow contracts both planes
    of a pair per instruction: lhsT = x8[128, 2, 128m], moving =
    w8[128, 2, <=256n] (moving free dim capped at 512).
  - A short burst of dummy matmuls warms the PE (HAM un-throttle) while the
    first activation tiles and weights stream in.
  - Host concatenates per-core outputs along N.
"""

import sys

for _p in ("/opt/trn_rl_repo", "/opt/pypackages"):
    if _p not in sys.path:
        sys.path.append(_p)

import numpy as np
import ml_dtypes

import concourse.bass as bass
import concourse.mybir as mybir
import concourse.tile as tile
from concourse import bacc

# Problem constants (hardcoded per harness contract)
B, S, K = 4, 2048, 4096
M = B * S                  # 8192 tokens
N = 11008                  # out features
NCORES = 8
NPC = N // NCORES          # per-core out features (1376)
P = 128
KPAIRS = K // (2 * P)      # 16 pairs of k-planes (256 k each)
CENTER = 7.5               # nibble centering; q - 7.5 is exact in e4m3


def _nchunks(npc, cw=256):
    return [(i, min(cw, npc - i)) for i in range(0, npc, cw)]


def build_nc(m=M, npc=NPC, mg=512, warmup=120, cw=512):
    """Build the per-core Bass program. m tokens, npc out cols, mg tokens per
    m-group (DMA granule), cw psum chunk width (512 f32 = one PSUM bank;
    DoubleRow moving free = 2*cw <= 1024)."""
    ngroups = m // mg
    mbs = mg // P              # m-blocks per group
    chunks = _nchunks(npc, cw)
    f8 = mybir.dt.float8e4

    nc = bacc.Bacc("TRN2", target_bir_lowering=False, debug=False)
    x8_d = nc.dram_tensor("x8", [KPAIRS, P, 2, m], f8, kind="ExternalInput")
    w8_d = nc.dram_tensor("w8", [KPAIRS, P, 2, npc], f8, kind="ExternalInput")
    s_d = nc.dram_tensor("s32", [P, npc], mybir.dt.float32, kind="ExternalInput")
    cb_d = nc.dram_tensor("cb32", [P, npc], mybir.dt.float32, kind="ExternalInput")
    rs_d = nc.dram_tensor("rs", [P, m // P], mybir.dt.float32, kind="ExternalInput")
    out_d = nc.dram_tensor("out", [m, npc], mybir.dt.float32, kind="ExternalOutput")

    with tile.TileContext(nc) as tc:
        with (
            tc.tile_pool(name="const", bufs=1) as const_pool,
            tc.tile_pool(name="w", bufs=1) as w_pool,
            tc.tile_pool(name="x", bufs=2) as x_pool,
            tc.tile_pool(name="o", bufs=2) as o_pool,
            tc.tile_pool(name="ps", bufs=2, space="PSUM") as ps_pool,
            tc.tile_pool(name="wps", bufs=1, space="PSUM") as warm_ps_pool,
        ):
            s32t = const_pool.tile([P, npc], mybir.dt.float32, tag="s32t")
            cb32t = const_pool.tile([P, npc], mybir.dt.float32, tag="cb32t")
            rs_t = const_pool.tile([P, m // P], mybir.dt.float32, tag="rs_t")
            # PE warmup: flip the HAM clock gate to 8/8 while DMAs run.
            if warmup:
                wsrc = const_pool.tile([P, 256], mybir.dt.bfloat16, tag="wsrc")
                nc.vector.memset(wsrc[:], 0.0)
                wp = warm_ps_pool.tile([P, 256], mybir.dt.float32, tag="wp")
                for _ in range(warmup):
                    nc.tensor.matmul(wp[:], wsrc[:, :P], wsrc[:], start=True, stop=True)

            xg0 = x_pool.tile([P, KPAIRS, 2, mg], f8, tag="xg")

            # Resident fp8 weight tiles, one per k-pair, already centered on host.
            w_tiles = [
                w_pool.tile([P, 2, npc], f8, name=f"W{t}", tag=f"W{t}")
                for t in range(KPAIRS)
            ]
            # interleave weight and first-group activation DMAs so pair t of
            # both lands early, letting mb0's accumulation start ASAP
            for t in range(KPAIRS):
                nc.sync.dma_start(w_tiles[t][:], w8_d[t])
                nc.sync.dma_start(xg0[:, t, :, :], x8_d[t, :, :, 0:mg])

            # scale rows are first needed at the first eviction: keep their
            # 1.4MB out of the pre-mb0 DMA critical path
            nc.sync.dma_start(s32t[:], s_d[:])
            nc.sync.dma_start(cb32t[:], cb_d[:])
            nc.sync.dma_start(rs_t[:], rs_d[:])

            # Main matmul loop: m-groups of `mg` tokens, 128-token m-blocks.
            for g in range(ngroups):
                if g == 0:
                    xg = xg0
                else:
                    xg = x_pool.tile([P, KPAIRS, 2, mg], f8, tag="xg")
                    for t in range(KPAIRS):
                        nc.sync.dma_start(
                            xg[:, t, :, :], x8_d[t, :, :, g * mg:(g + 1) * mg]
                        )
                for mb in range(mbs):
                    mbi = g * mbs + mb
                    ps = ps_pool.tile([P, npc], mybir.dt.float32, tag="ps")
                    for t in range(KPAIRS):
                        lhsT = xg[:, t, :, mb * P:(mb + 1) * P]
                        for (n0, nw) in chunks:
                            # start=True marks the whole 2KB PSUM bank (the
                            # zero region) pending-zero, so only a chunk that
                            # begins a bank may issue it; a bank's later
                            # chunks inherit the marking and their first
                            # (start=False) write still overwrites.
                            bank_first = (n0 % 512) == 0
                            nc.tensor.matmul(
                                ps[:, n0:n0 + nw], lhsT,
                                w_tiles[t][:, :, n0:n0 + nw],
                                start=(t == 0 and bank_first),
                                stop=(t == KPAIRS - 1),
                                perf_mode=mybir.MatmulPerfMode.DoubleRow,
                                skip_group_check=(not bank_first),
                            )
                    # eviction (ACT), then dequant on SBUF:
                    #   out = psum * s + s*(7.5 - z) * rowsum[m]
                    ot = o_pool.tile([P, npc], mybir.dt.float32, tag="ot")
                    nc.scalar.copy(ot[:], ps[:])
                    nc.vector.tensor_tensor(
                        ot[:], ot[:], s32t[:], op=mybir.AluOpType.mult
                    )
                    nc.vector.scalar_tensor_tensor(
                        ot[:], cb32t[:], rs_t[:, mbi:mbi + 1], ot[:],
                        op0=mybir.AluOpType.mult, op1=mybir.AluOpType.add,
                    )
                    m0 = g * mg + mb * P
                    nc.sync.dma_start(out_d[m0:m0 + P, :], ot[:])

    nc.compile()
    return nc


def prep_inputs(inp, quant_weight, scales, zeros, ncores=NCORES, npc=NPC):
    """Host-side sharding/layout: returns in_maps list for run_bass_kernel_spmd."""
    m = inp.shape[0] * inp.shape[1]
    k = inp.shape[2]

    x = np.asarray(inp, dtype=np.float32).reshape(m, k)
    # x8[t, p, i, tok] = e4m3(x[tok, 256t + 2p + i]): plane i=0 even k (low
    # nibble), i=1 odd k (high nibble), paired per DoubleRow instruction
    x8 = np.ascontiguousarray(
        x.reshape(m, KPAIRS, P, 2).astype(ml_dtypes.float8_e4m3)
        .transpose(1, 2, 3, 0)
    )

    # rowsum of the exact activations, for the center/zero correction term
    rs = x.sum(axis=1, dtype=np.float64).astype(np.float32)  # [m]
    rs_host = np.ascontiguousarray(rs.reshape(m // P, P).T)  # [P, m//P]

    n = quant_weight.shape[0]
    assert n == ncores * npc, (n, ncores, npc)
    qw8 = np.asarray(quant_weight).astype(np.uint8)          # [N, k//2]
    lo = (qw8 & 15).astype(np.float32) - CENTER              # even k
    hi = (qw8 >> 4).astype(np.float32) - CENTER              # odd k
    s_all = np.asarray(scales, dtype=np.float32).reshape(-1)
    z_all = np.asarray(zeros, dtype=np.float32).reshape(-1)
    cb_all = s_all * (CENTER - z_all)

    in_maps = []
    for c in range(ncores):
        sl = slice(c * npc, (c + 1) * npc)
        # w8[t, p, i, n]: centered nibbles, exact in e4m3
        wc = np.stack([lo[sl].T, hi[sl].T], axis=1)          # [k//2, 2, npc]
        wc = np.ascontiguousarray(
            wc.reshape(KPAIRS, P, 2, npc).astype(ml_dtypes.float8_e4m3)
        )
        s_c = np.ascontiguousarray(np.broadcast_to(s_all[sl], (P, npc)))
        cb_c = np.ascontiguousarray(np.broadcast_to(cb_all[sl], (P, npc)))
        in_maps.append(
            {"x8": x8, "w8": wc, "s32": s_c, "cb32": cb_c, "rs": rs_host}
        )
    return in_maps


_NC_CACHE = {}


def _get_nc():
    if "nc" not in _NC_CACHE:
        _NC_CACHE["nc"] = build_nc()
    return _NC_CACHE["nc"]


def kernel(inp, quant_weight, scales, zeros):
    from concourse.bass_utils import run_bass_kernel_spmd

    nc = _get_nc()
    in_maps = prep_inputs(inp, quant_weight, scales, zeros)
    res = run_bass_kernel_spmd(nc, in_maps, list(range(NCORES)))
    out = np.concatenate([res.results[c]["out"] for c in range(NCORES)], axis=1)
    return np.ascontiguousarray(out).reshape(B, S, N)
